# revision 19
# baseline (speedup 1.0000x reference)
"""Trainium2 Bass kernel for nn_DocREModel (doc-level relation extraction graph pooling).

Key structure exploited: every use of `attention` reduces over heads first
(S = sum_h A[h]), and only a few rows/cols of S are ever read:
  - mention contexts need the 128 mention rows of S (full 1024 width),
  - link-span pooling w = (mask @ S) * mask only touches S[span_rows x span_cols]
    per span (and span_cols == span_rows as index sets).
So the host ships, per doc, only the gathered slices of attention (uint8
quantized x255), and the device does the O(big) work: the 12-head sum,
the S-row contractions against seq, and the span mask/pool matmuls.

Sharding: 2 cores per doc (B=4 -> 8 cores):
  - mention path split by S-columns: core g of doc b handles columns
    [512g, 512g+512); host sums the two partial numerator outputs.
  - link path split by spans: core g handles spans [8g, 8g+8) entirely
    (its own 256-row span block; 8*31 = 248 <= 256 always fits); host
    concatenates the two v outputs.
Host applies the tiny normalizations (epsilon divides, entity pooling,
logsumexp, type concat) on the small results.

Per-core device inputs (host-prepped: index gather + transpose + u8 cast only):
  att_c [128, 12*1024] u8: [p, h, 0:512]   = span block [rq(2),c(256)]
                            [p, h, 512:1024]= mention rows transposed [rq(4),m(128)]
  seq   [128, 4*770] bf16: seq_aug[g*512+rq*128+p, :] as [p, rq, 770]
  seqg  [128, 2*770] bf16: seq_aug[row_g[cc*128+p], :] as [p, cc, 770]
  msb   [128, 2*8]   bf16: span-row mask msb_g[rq*128+p, k] as [p, rq, 8]
  mcc   [128, 2*8]   bf16: msb_g[cc*128+p, k] as [p, cc, 8]
Outputs (all x255 scaled): out_m0 [128, 512] + out_m1 [128, 258] f32
  (mention-context numerators; col 768 = row-sum), out_v [8, 770] f32.
"""

import os
import sys

for _p in ("/opt/trn_rl_repo", "/root/.axon_site/_ro/trn_rl_repo"):
    if os.path.isdir(_p) and _p not in sys.path:
        sys.path.insert(0, _p)

import numpy as np

B, L, H, NH = 4, 1024, 768, 12
E, MPE, K = 32, 4, 16
EM = E * MPE              # 128 mentions per doc
KH = K // 2               # spans per core
TYPE_DIM = 20
OFFSET = 1
SBH = 256                 # per-core span block (8 spans x <=31 rows <= 248)
HA = H + 2                # 768 + row-sum ones col + even pad
N1 = 512                  # PSUM bank split of the 770-wide outputs
QS = 255.0                # uint8 quantization scale for attention


def _build_nc(debug=False):
    import concourse.bass as bass
    import concourse.mybir as mybir
    import concourse.tile as tile
    from concourse import bacc

    f32 = mybir.dt.float32
    bf16 = mybir.dt.bfloat16
    u8 = mybir.dt.uint8

    nc = bacc.Bacc("TRN2", target_bir_lowering=False, debug=debug)

    att_c = nc.dram_tensor("att_c", [128, 12 * 1024], u8, kind="ExternalInput")
    seq = nc.dram_tensor("seq", [128, 4 * HA], bf16, kind="ExternalInput")
    seqg = nc.dram_tensor("seqg", [128, 2 * HA], bf16, kind="ExternalInput")
    msb = nc.dram_tensor("msb", [128, 2 * KH], bf16, kind="ExternalInput")
    mcc = nc.dram_tensor("mcc", [128, 2 * KH], bf16, kind="ExternalInput")
    out_m = nc.dram_tensor("out_m", [EM, HA], bf16, kind="ExternalOutput")
    out_v = nc.dram_tensor("out_v", [KH, HA], bf16, kind="ExternalOutput")

    with tile.TileContext(nc) as tc:
        with (
            tc.tile_pool(name="const", bufs=1) as constp,
            tc.tile_pool(name="att", bufs=1) as attp,
            tc.tile_pool(name="work", bufs=1) as workp,
            tc.tile_pool(name="ps", bufs=8, space="PSUM") as psp,
        ):
            # ---- SBUF tiles ----
            att_t = attp.tile([128, 12, 1024], u8, tag="attc", name="attc")
            seq_t = constp.tile([128, 4, HA], bf16, tag="seq", name="seq")
            seqg_t = constp.tile([128, 2, HA], bf16, tag="seqg", name="seqg")
            msb_t = constp.tile([128, 2, KH], bf16, tag="msb", name="msb")
            mcc_t = constp.tile([128, 2, KH], bf16, tag="mcc", name="mcc")

            # ---- input DMAs: head-pair transfers. DVE consumes pairs 2-5, so
            #      those lead both HW DGE queues; pool's pairs (0, 1) land last
            #      among the attention pairs, consts after. ----
            for i in (2, 4, 0):
                nc.sync.dma_start(out=att_t[:, 2 * i:2 * i + 2, :],
                                  in_=att_c[:, 2048 * i:2048 * (i + 1)])
            for i in (3, 5, 1):
                nc.scalar.dma_start(out=att_t[:, 2 * i:2 * i + 2, :],
                                    in_=att_c[:, 2048 * i:2048 * (i + 1)])
            nc.scalar.dma_start(out=msb_t[:], in_=msb[:])
            nc.scalar.dma_start(out=mcc_t[:], in_=mcc[:])
            nc.scalar.dma_start(out=seqg_t[:], in_=seqg[:])
            nc.scalar.dma_start(out=seq_t[:], in_=seq[:])

            # ---- 12-head sum as two trees over [128, 512] slabs:
            #      span slab = att_t[:, h, 0:512] (rq2 x c256),
            #      mention slab = att_t[:, h, 512:1024] (rq4 x m128).
            #      Span tree finishes first so the GTT/v chain on PE overlaps
            #      the mention tree. gpsimd sums the two late pairs (both
            #      halves); their contributions join each tree last. ----
            S_c = workp.tile([128, 1024], bf16, tag="sc", name="sc")
            p0 = workp.tile([128, 1024], bf16, tag="p0", name="p0")
            p1 = workp.tile([128, 1024], bf16, tag="p1", name="p1")
            p2 = workp.tile([128, 1024], bf16, tag="p2", name="p2")
            p3 = workp.tile([128, 1024], bf16, tag="p3", name="p3")
            p4 = workp.tile([128, 1024], bf16, tag="p4", name="p4")
            p5 = workp.tile([128, 1024], bf16, tag="p5", name="p5")
            SP, MN = slice(0, 512), slice(512, 1024)

            # DVE span tree: pairs 2..5 first (land first), 0..1 join last
            nc.vector.tensor_add(p2[:, SP], att_t[:, 4, SP], att_t[:, 5, SP])
            nc.vector.tensor_add(p3[:, SP], att_t[:, 6, SP], att_t[:, 7, SP])
            nc.vector.tensor_add(p4[:, SP], att_t[:, 8, SP], att_t[:, 9, SP])
            nc.vector.tensor_add(p5[:, SP], att_t[:, 10, SP], att_t[:, 11, SP])
            nc.vector.tensor_add(p2[:, SP], p2[:, SP], p3[:, SP])
            nc.vector.tensor_add(p4[:, SP], p4[:, SP], p5[:, SP])
            nc.vector.tensor_add(p0[:, SP], att_t[:, 0, SP], att_t[:, 1, SP])
            nc.vector.tensor_add(p1[:, SP], att_t[:, 2, SP], att_t[:, 3, SP])
            nc.vector.tensor_add(p2[:, SP], p2[:, SP], p4[:, SP])
            nc.vector.tensor_add(p0[:, SP], p0[:, SP], p1[:, SP])
            nc.vector.tensor_add(S_c[:, SP], p2[:, SP], p0[:, SP])
            # DVE mention tree
            nc.vector.tensor_add(p2[:, MN], att_t[:, 4, MN], att_t[:, 5, MN])
            nc.vector.tensor_add(p3[:, MN], att_t[:, 6, MN], att_t[:, 7, MN])
            nc.vector.tensor_add(p4[:, MN], att_t[:, 8, MN], att_t[:, 9, MN])
            nc.vector.tensor_add(p5[:, MN], att_t[:, 10, MN], att_t[:, 11, MN])
            nc.vector.tensor_add(p2[:, MN], p2[:, MN], p3[:, MN])
            nc.vector.tensor_add(p4[:, MN], p4[:, MN], p5[:, MN])
            nc.vector.tensor_add(p0[:, MN], att_t[:, 0, MN], att_t[:, 1, MN])
            nc.vector.tensor_add(p1[:, MN], att_t[:, 2, MN], att_t[:, 3, MN])
            nc.vector.tensor_add(p2[:, MN], p2[:, MN], p4[:, MN])
            nc.vector.tensor_add(p0[:, MN], p0[:, MN], p1[:, MN])
            nc.vector.tensor_add(S_c[:, MN], p2[:, MN], p0[:, MN])

            # ---- GTmask^T[c', k] = sum_r S_span[r, c'] * msb[r, k], acc over rq ----
            ps_g0 = psp.tile([128, KH], f32, tag="ps", name="ps_g0")
            ps_g1 = psp.tile([128, KH], f32, tag="ps", name="ps_g1")
            for rc in range(2):
                nc.tensor.matmul(ps_g0[:], S_c[:, 256 * rc:256 * rc + 128],
                                 msb_t[:, rc, :], start=(rc == 0), stop=(rc == 1))
                nc.tensor.matmul(ps_g1[:], S_c[:, 256 * rc + 128:256 * rc + 256],
                                 msb_t[:, rc, :], start=(rc == 0), stop=(rc == 1))

            # ---- w^T = GTmask^T * mask (zero outside own span cols) ----
            wssT = workp.tile([128, 2, KH], bf16, tag="wssT", name="wssT")
            nc.vector.tensor_mul(wssT[:, 0, :], ps_g0[:], mcc_t[:, 0, :])
            nc.vector.tensor_mul(wssT[:, 1, :], ps_g1[:], mcc_t[:, 1, :])

            # ---- link numerators v[k, :] = sum_c' w^T[c', k] seqg[c', :] ----
            ps_v0 = psp.tile([KH, N1], f32, tag="ps", name="ps_v0")
            ps_v1 = psp.tile([KH, HA - N1], f32, tag="ps", name="ps_v1")
            for cc in range(2):
                nc.tensor.matmul(ps_v0[:], wssT[:, cc, :], seqg_t[:, cc, 0:N1],
                                 start=(cc == 0), stop=(cc == 1))
                nc.tensor.matmul(ps_v1[:], wssT[:, cc, :], seqg_t[:, cc, N1:HA],
                                 start=(cc == 0), stop=(cc == 1))
            v_t = workp.tile([KH, HA], bf16, tag="v", name="v")
            nc.scalar.copy(out=v_t[:, 0:N1], in_=ps_v0[:])
            nc.vector.tensor_copy(v_t[:, N1:HA], ps_v1[:])
            nc.scalar.dma_start(out=out_v[:], in_=v_t[:])

            # ---- mention-context numerators mnum[m, :] = sum_c S[pos_m, c] seq[c, :];
            #      N-halves in separate loops so copy/DMA of the first half
            #      overlaps the second half's matmuls ----
            ps_m0 = psp.tile([EM, N1], f32, tag="ps", name="ps_m0")
            ps_m1 = psp.tile([EM, HA - N1], f32, tag="ps", name="ps_m1")
            m_t = workp.tile([EM, HA], bf16, tag="mout", name="mout")
            for rc in range(4):
                nc.tensor.matmul(ps_m1[:], S_c[:, 512 + 128 * rc:512 + 128 * (rc + 1)],
                                 seq_t[:, rc, N1:HA], start=(rc == 0), stop=(rc == 3))
            nc.vector.tensor_copy(m_t[:, N1:HA], ps_m1[:])
            for rc in range(4):
                nc.tensor.matmul(ps_m0[:], S_c[:, 512 + 128 * rc:512 + 128 * (rc + 1)],
                                 seq_t[:, rc, 0:N1], start=(rc == 0), stop=(rc == 3))
            nc.scalar.copy(out=m_t[:, 0:N1], in_=ps_m0[:])
            nc.scalar.dma_start(out=out_m[:], in_=m_t[:])

    nc.compile()
    return nc


_NC_CACHE = {}


def _get_nc():
    if "nc" not in _NC_CACHE:
        _NC_CACHE["nc"] = _build_nc()
    return _NC_CACHE["nc"]


def _per_core_inputs(sequence_output, attention, mention_pos, link_start, link_len):
    """Host prep: index gathers, transposes, u8 quantize. Returns (in_maps, per-doc
    (pos, lengths) for the combine step)."""
    import ml_dtypes
    seq = np.asarray(sequence_output, dtype=np.float32)
    att = np.asarray(attention, dtype=np.float32)
    mpos = np.asarray(mention_pos).astype(np.int64)
    lstart = np.asarray(link_start).astype(np.int64)
    llen = np.asarray(link_len).astype(np.int64)
    bf = ml_dtypes.bfloat16

    in_maps = []
    metas = []
    for b in range(B):
        pos = (mpos[b] + OFFSET).reshape(EM)
        s = lstart[b] + OFFSET
        e = lstart[b] + llen[b] + 1 + OFFSET
        att_b = att[b]                                            # [12, 1024, 1024]
        seq_aug = np.concatenate(
            [seq[b], np.ones((L, 1), np.float32), np.zeros((L, 1), np.float32)],
            axis=1)                                               # [1024, 770]
        # mention rows, quantized
        am = att_b[:, pos, :]                                     # [12, 128, 1024]
        amq = np.minimum(np.round(am * QS), 255).astype(np.uint8)

        for g in range(2):
            ks = slice(KH * g, KH * (g + 1))
            rowset = sorted(set(int(r) for k in range(KH * g, KH * (g + 1))
                                for r in range(s[k], e[k])))
            nsp = len(rowset)
            assert nsp <= SBH
            rowarr = np.zeros(SBH, np.int64)
            rowarr[:nsp] = rowset
            valid = (np.arange(SBH) < nsp)[:, None]
            msb_g = ((rowarr[:, None] >= s[None, ks]) & (rowarr[:, None] < e[None, ks])
                     & valid).astype(np.float32)                  # [256, 8]

            # span block rows x cols both = rowarr, quantized
            ss = att_b[:, rowarr, :][:, :, rowarr]                # [12, 256, 256]
            ssq = np.minimum(np.round(ss * QS), 255).astype(np.uint8)
            # span part [p, h, rq(2), c(256)]
            sp = ssq.reshape(12, 2, 128, 256).transpose(2, 0, 1, 3)
            # mention part [p, h, rq(4), m(128)], c = g*512 + rq*128 + p
            a = amq[:, :, g * 512:(g + 1) * 512]                  # [12, 128m, 512c]
            mp = a.reshape(12, 128, 4, 128).transpose(3, 0, 2, 1)
            att_in = np.concatenate(
                [sp.reshape(128, 12, 512), mp.reshape(128, 12, 512)],
                axis=2)                                           # [p, h, 1024]
            att_in = np.ascontiguousarray(att_in).reshape(128, 12 * 1024)
            # seq rows for this core's mention column half
            sq = seq_aug[g * 512:(g + 1) * 512]                   # [512, 770]
            sq = sq.reshape(4, 128, HA).transpose(1, 0, 2)        # [p, rq, 770]
            seq_in = np.ascontiguousarray(sq.astype(bf)).reshape(128, 4 * HA)
            # seqg rows at this core's span block positions
            sg = seq_aug[rowarr]                                  # [256, 770]
            sg = sg.reshape(2, 128, HA).transpose(1, 0, 2)        # [p, cc, 770]
            seqg_in = np.ascontiguousarray(sg.astype(bf)).reshape(128, 2 * HA)
            # masks [p, rq/cc, 8]
            mq = msb_g.reshape(2, 128, KH).transpose(1, 0, 2)
            msb_in = np.ascontiguousarray(mq.astype(bf)).reshape(128, 2 * KH)
            mcc_in = msb_in.copy()
            in_maps.append({"att_c": att_in, "seq": seq_in, "seqg": seqg_in,
                            "msb": msb_in, "mcc": mcc_in})
        metas.append((pos, (e - s).astype(np.float32)))
    return in_maps, metas


def _combine(outs, metas, sequence_output, type_table):
    seq = np.asarray(sequence_output, dtype=np.float32)
    ttab = np.asarray(type_table, dtype=np.float32)
    type_ids = np.concatenate(
        [np.zeros(E, np.int64), np.ones(EM, np.int64), np.full(K, 2, np.int64)])
    nodes_type = ttab[type_ids]                                   # [176, 20]

    out = np.zeros((B, E + EM + K + E + EM, H + TYPE_DIM), np.float32)
    for b in range(B):
        pos, length = metas[b]
        o0, o1 = outs[2 * b], outs[2 * b + 1]
        mnum = o0["out_m"].astype(np.float32) + o1["out_m"].astype(np.float32)
        v = np.concatenate([o0["out_v"], o1["out_v"]],
                           axis=0).astype(np.float32)             # [16, 770], x255

        m_ctx = mnum[:, :H] / (mnum[:, H:H + 1] + QS * NH * 1e-5)
        enum = mnum.reshape(E, MPE, HA).sum(axis=1)
        e_ctx = enum[:, :H] / (enum[:, H:H + 1] + QS * NH * MPE * 1e-5)
        link_rep = v[:, :H] / (QS * NH * length[:, None])

        memb = seq[b][pos]                                        # [128, 768] exact
        mg = memb.reshape(E, MPE, H)
        mmax = mg.max(axis=1)
        eemb = np.log(np.exp(mg - mmax[:, None, :]).sum(axis=1)) + mmax

        nodes_raw = np.concatenate([eemb, memb, link_rep], axis=0)    # [176, H]
        nodes = np.concatenate([nodes_raw, nodes_type], axis=1)       # [176, H+20]
        ctx = np.concatenate([e_ctx, m_ctx], axis=0)                  # [160, H]
        ctx = np.concatenate([ctx, np.zeros((E + EM, TYPE_DIM), np.float32)], axis=1)
        out[b] = np.concatenate([nodes, ctx], axis=0)
    return out


def kernel(**inputs):
    from concourse.bass_utils import run_bass_kernel_spmd

    in_maps, metas = _per_core_inputs(
        inputs["sequence_output"], inputs["attention"],
        inputs["mention_pos"], inputs["link_start"], inputs["link_len"])
    nc = _get_nc()
    res = run_bass_kernel_spmd(nc, in_maps, core_ids=list(range(8)))
    return _combine(res.results, metas, inputs["sequence_output"], inputs["type_table"])


# revision 23
# speedup vs baseline: 1.0102x; 1.0102x over previous
"""Trainium2 Bass kernel for nn_DocREModel (doc-level relation extraction graph pooling).

Key structure exploited: every use of `attention` reduces over heads first
(S = sum_h A[h]), and only a few rows/cols of S are ever read:
  - mention contexts need the 128 mention rows of S (full 1024 width),
  - link-span pooling w = (mask @ S) * mask only touches S[span_rows x span_cols]
    per span (and span_cols == span_rows as index sets).
So the host ships, per doc, only the gathered slices of attention (uint8
quantized x255), and the device does the O(big) work: the 12-head sum,
the S-row contractions against seq, and the span mask/pool matmuls.

Sharding: 2 cores per doc (B=4 -> 8 cores):
  - mention path split by S-columns: core g of doc b handles columns
    [512g, 512g+512); host sums the two partial numerator outputs.
  - link path split by spans: core g handles spans [8g, 8g+8) entirely
    (its own 256-row span block; 8*31 = 248 <= 256 always fits); host
    concatenates the two v outputs.
Host applies the tiny normalizations (epsilon divides, entity pooling,
logsumexp, type concat) on the small results.

Per-core device inputs (host-prepped: index gather + transpose + u8 cast only):
  att_c [128, 12*1024] u8: [p, h, 0:512]   = span block [rq(2),c(256)]
                            [p, h, 512:1024]= mention rows transposed [rq(4),m(128)]
  seq   [128, 4*770] bf16: seq_aug[g*512+rq*128+p, :] as [p, rq, 770]
  seqg  [128, 2*770] bf16: seq_aug[row_g[cc*128+p], :] as [p, cc, 770]
  msb   [128, 2*8]   bf16: span-row mask msb_g[rq*128+p, k] as [p, rq, 8]
  mcc   [128, 2*8]   bf16: msb_g[cc*128+p, k] as [p, cc, 8]
Outputs (all x255 scaled): out_m0 [128, 512] + out_m1 [128, 258] f32
  (mention-context numerators; col 768 = row-sum), out_v [8, 770] f32.
"""

import os
import sys

for _p in ("/opt/trn_rl_repo", "/root/.axon_site/_ro/trn_rl_repo"):
    if os.path.isdir(_p) and _p not in sys.path:
        sys.path.insert(0, _p)

import numpy as np

B, L, H, NH = 4, 1024, 768, 12
E, MPE, K = 32, 4, 16
EM = E * MPE              # 128 mentions per doc
KH = K // 2               # spans per core
TYPE_DIM = 20
OFFSET = 1
SBH = 256                 # per-core span block (8 spans x <=31 rows <= 248)
HA = H + 2                # 768 + row-sum ones col + even pad
N1 = 512                  # PSUM bank split of the 770-wide outputs
QS = 255.0                # uint8 quantization scale for attention


def _build_nc(debug=False):
    import concourse.bass as bass
    import concourse.mybir as mybir
    import concourse.tile as tile
    from concourse import bacc

    f32 = mybir.dt.float32
    bf16 = mybir.dt.bfloat16
    u8 = mybir.dt.uint8

    nc = bacc.Bacc("TRN2", target_bir_lowering=False, debug=debug)

    att_c = nc.dram_tensor("att_c", [128, 8 * 1024], u8, kind="ExternalInput")
    att_b = nc.dram_tensor("att_b", [128, 4 * 1024], bf16, kind="ExternalInput")
    seq = nc.dram_tensor("seq", [128, 4 * HA], bf16, kind="ExternalInput")
    seqg = nc.dram_tensor("seqg", [128, 2 * HA], bf16, kind="ExternalInput")
    msb = nc.dram_tensor("msb", [128, 2 * KH], bf16, kind="ExternalInput")
    mcc = nc.dram_tensor("mcc", [128, 2 * KH], bf16, kind="ExternalInput")
    out_m = nc.dram_tensor("out_m", [EM, HA], bf16, kind="ExternalOutput")
    out_v = nc.dram_tensor("out_v", [KH, HA], bf16, kind="ExternalOutput")

    with tile.TileContext(nc) as tc:
        with (
            tc.tile_pool(name="const", bufs=1) as constp,
            tc.tile_pool(name="att", bufs=1) as attp,
            tc.tile_pool(name="work", bufs=1) as workp,
            tc.tile_pool(name="ps", bufs=8, space="PSUM") as psp,
        ):
            # ---- SBUF tiles ----
            att_t = attp.tile([128, 8, 1024], u8, tag="attc", name="attc")
            atb_t = attp.tile([128, 4, 1024], bf16, tag="attb", name="attb")
            seq_t = constp.tile([128, 4, HA], bf16, tag="seq", name="seq")
            seqg_t = constp.tile([128, 2, HA], bf16, tag="seqg", name="seqg")
            msb_t = constp.tile([128, 2, KH], bf16, tag="msb", name="msb")
            mcc_t = constp.tile([128, 2, KH], bf16, tag="mcc", name="mcc")

            # ---- input DMAs: head-pair transfers. DVE consumes pairs 2-5, so
            #      those lead both HW DGE queues; pool's pairs (0, 1) land last
            #      among the attention pairs, consts after. ----
            nc.sync.dma_start(out=atb_t[:, 0:2, :], in_=att_b[:, 0:2048])
            nc.scalar.dma_start(out=atb_t[:, 2:4, :], in_=att_b[:, 2048:4096])
            for i, q in ((2, nc.sync), (3, nc.scalar), (0, nc.sync), (1, nc.scalar)):
                q.dma_start(out=att_t[:, 2 * i:2 * i + 2, :],
                            in_=att_c[:, 2048 * i:2048 * (i + 1)])
            nc.scalar.dma_start(out=msb_t[:], in_=msb[:])
            nc.scalar.dma_start(out=mcc_t[:], in_=mcc[:])
            nc.scalar.dma_start(out=seqg_t[:], in_=seqg[:])
            nc.scalar.dma_start(out=seq_t[:], in_=seq[:])

            # ---- 12-head sum as two trees over [128, 512] slabs:
            #      span slab = att_t[:, h, 0:512] (rq2 x c256),
            #      mention slab = att_t[:, h, 512:1024] (rq4 x m128).
            #      Span tree finishes first so the GTT/v chain on PE overlaps
            #      the mention tree. gpsimd sums the two late pairs (both
            #      halves); their contributions join each tree last. ----
            S_c = workp.tile([128, 1024], bf16, tag="sc", name="sc")
            p0 = workp.tile([128, 1024], bf16, tag="p0", name="p0")
            p1 = workp.tile([128, 1024], bf16, tag="p1", name="p1")
            p2 = workp.tile([128, 1024], bf16, tag="p2", name="p2")
            p3 = workp.tile([128, 1024], bf16, tag="p3", name="p3")
            p4 = workp.tile([128, 1024], bf16, tag="p4", name="p4")
            p5 = workp.tile([128, 1024], bf16, tag="p5", name="p5")
            SP, MN = slice(0, 512), slice(512, 1024)

            # DVE span tree: pairs 2..5 first (land first), 0..1 join last
            nc.vector.tensor_add(p2[:, SP], atb_t[:, 0, SP], atb_t[:, 1, SP])
            nc.vector.tensor_add(p3[:, SP], atb_t[:, 2, SP], atb_t[:, 3, SP])
            nc.vector.tensor_add(p4[:, SP], att_t[:, 4, SP], att_t[:, 5, SP])
            nc.vector.tensor_add(p5[:, SP], att_t[:, 6, SP], att_t[:, 7, SP])
            nc.vector.tensor_add(p2[:, SP], p2[:, SP], p3[:, SP])
            nc.vector.tensor_add(p4[:, SP], p4[:, SP], p5[:, SP])
            nc.vector.tensor_add(p0[:, SP], att_t[:, 0, SP], att_t[:, 1, SP])
            nc.vector.tensor_add(p1[:, SP], att_t[:, 2, SP], att_t[:, 3, SP])
            nc.vector.tensor_add(p2[:, SP], p2[:, SP], p4[:, SP])
            nc.vector.tensor_add(p0[:, SP], p0[:, SP], p1[:, SP])
            nc.vector.tensor_add(S_c[:, SP], p2[:, SP], p0[:, SP])
            # ---- GTmask^T[c', k] = sum_r S_span[r, c'] * msb[r, k], acc over rq ----
            ps_g0 = psp.tile([128, KH], f32, tag="ps", name="ps_g0")
            ps_g1 = psp.tile([128, KH], f32, tag="ps", name="ps_g1")
            for rc in range(2):
                nc.tensor.matmul(ps_g0[:], S_c[:, 256 * rc:256 * rc + 128],
                                 msb_t[:, rc, :], start=(rc == 0), stop=(rc == 1))
                nc.tensor.matmul(ps_g1[:], S_c[:, 256 * rc + 128:256 * rc + 256],
                                 msb_t[:, rc, :], start=(rc == 0), stop=(rc == 1))

            # ---- w^T = GTmask^T * mask (zero outside own span cols) ----
            wssT = workp.tile([128, 2, KH], bf16, tag="wssT", name="wssT")
            nc.vector.tensor_mul(wssT[:, 0, :], ps_g0[:], mcc_t[:, 0, :])
            nc.vector.tensor_mul(wssT[:, 1, :], ps_g1[:], mcc_t[:, 1, :])
            # DVE mention tree
            nc.vector.tensor_add(p2[:, MN], atb_t[:, 0, MN], atb_t[:, 1, MN])
            nc.vector.tensor_add(p3[:, MN], atb_t[:, 2, MN], atb_t[:, 3, MN])
            nc.vector.tensor_add(p4[:, MN], att_t[:, 4, MN], att_t[:, 5, MN])
            nc.vector.tensor_add(p5[:, MN], att_t[:, 6, MN], att_t[:, 7, MN])
            nc.vector.tensor_add(p2[:, MN], p2[:, MN], p3[:, MN])
            nc.vector.tensor_add(p4[:, MN], p4[:, MN], p5[:, MN])
            nc.vector.tensor_add(p0[:, MN], att_t[:, 0, MN], att_t[:, 1, MN])
            nc.vector.tensor_add(p1[:, MN], att_t[:, 2, MN], att_t[:, 3, MN])
            nc.vector.tensor_add(p2[:, MN], p2[:, MN], p4[:, MN])
            nc.vector.tensor_add(p0[:, MN], p0[:, MN], p1[:, MN])
            nc.vector.tensor_add(S_c[:, MN], p2[:, MN], p0[:, MN])


            # ---- link numerators v[k, :] = sum_c' w^T[c', k] seqg[c', :] ----
            ps_v0 = psp.tile([KH, N1], f32, tag="ps", name="ps_v0")
            ps_v1 = psp.tile([KH, HA - N1], f32, tag="ps", name="ps_v1")
            for cc in range(2):
                nc.tensor.matmul(ps_v0[:], wssT[:, cc, :], seqg_t[:, cc, 0:N1],
                                 start=(cc == 0), stop=(cc == 1))
                nc.tensor.matmul(ps_v1[:], wssT[:, cc, :], seqg_t[:, cc, N1:HA],
                                 start=(cc == 0), stop=(cc == 1))
            v_t = workp.tile([KH, HA], bf16, tag="v", name="v")
            nc.scalar.copy(out=v_t[:, 0:N1], in_=ps_v0[:])
            nc.vector.tensor_copy(v_t[:, N1:HA], ps_v1[:])
            nc.scalar.dma_start(out=out_v[:], in_=v_t[:])

            # ---- mention-context numerators mnum[m, :] = sum_c S[pos_m, c] seq[c, :];
            #      N-halves in separate loops so copy/DMA of the first half
            #      overlaps the second half's matmuls ----
            ps_m0 = psp.tile([EM, N1], f32, tag="ps", name="ps_m0")
            ps_m1 = psp.tile([EM, HA - N1], f32, tag="ps", name="ps_m1")
            m_t = workp.tile([EM, HA], bf16, tag="mout", name="mout")
            for rc in range(4):
                nc.tensor.matmul(ps_m1[:], S_c[:, 512 + 128 * rc:512 + 128 * (rc + 1)],
                                 seq_t[:, rc, N1:HA], start=(rc == 0), stop=(rc == 3))
            nc.vector.tensor_copy(m_t[:, N1:HA], ps_m1[:])
            for rc in range(4):
                nc.tensor.matmul(ps_m0[:], S_c[:, 512 + 128 * rc:512 + 128 * (rc + 1)],
                                 seq_t[:, rc, 0:N1], start=(rc == 0), stop=(rc == 3))
            nc.scalar.copy(out=m_t[:, 0:N1], in_=ps_m0[:])
            nc.scalar.dma_start(out=out_m[:], in_=m_t[:])

    nc.compile()
    return nc


_NC_CACHE = {}


def _get_nc():
    if "nc" not in _NC_CACHE:
        _NC_CACHE["nc"] = _build_nc()
    return _NC_CACHE["nc"]


def _per_core_inputs(sequence_output, attention, mention_pos, link_start, link_len):
    """Host prep: index gathers, transposes, u8 quantize. Returns (in_maps, per-doc
    (pos, lengths) for the combine step)."""
    import ml_dtypes
    seq = np.asarray(sequence_output, dtype=np.float32)
    att = np.asarray(attention, dtype=np.float32)
    mpos = np.asarray(mention_pos).astype(np.int64)
    lstart = np.asarray(link_start).astype(np.int64)
    llen = np.asarray(link_len).astype(np.int64)
    bf = ml_dtypes.bfloat16

    in_maps = []
    metas = []
    for b in range(B):
        pos = (mpos[b] + OFFSET).reshape(EM)
        s = lstart[b] + OFFSET
        e = lstart[b] + llen[b] + 1 + OFFSET
        att_doc = att[b]                                          # [12, 1024, 1024]
        seq_aug = np.concatenate(
            [seq[b], np.ones((L, 1), np.float32), np.zeros((L, 1), np.float32)],
            axis=1)                                               # [1024, 770]
        am = att_doc[:, pos, :] * QS                              # [12, 128, 1024]

        for g in range(2):
            ks = slice(KH * g, KH * (g + 1))
            rowset = sorted(set(int(r) for k in range(KH * g, KH * (g + 1))
                                for r in range(s[k], e[k])))
            nsp = len(rowset)
            assert nsp <= SBH
            rowarr = np.zeros(SBH, np.int64)
            rowarr[:nsp] = rowset
            valid = (np.arange(SBH) < nsp)[:, None]
            msb_g = ((rowarr[:, None] >= s[None, ks]) & (rowarr[:, None] < e[None, ks])
                     & valid).astype(np.float32)                  # [256, 8]

            # span block rows x cols both = rowarr, x255
            ss = att_doc[:, rowarr, :][:, :, rowarr] * QS         # [12, 256, 256]
            # span part [p, h, rq(2), c(256)]
            sp = ss.reshape(12, 2, 128, 256).transpose(2, 0, 1, 3)
            # mention part [p, h, rq(4), m(128)], c = g*512 + rq*128 + p
            a = am[:, :, g * 512:(g + 1) * 512]                   # [12, 128m, 512c]
            mp = a.reshape(12, 128, 4, 128).transpose(3, 0, 2, 1)
            comb = np.concatenate(
                [sp.reshape(128, 12, 512), mp.reshape(128, 12, 512)],
                axis=2)                                           # [p, h, 1024] f32
            # heads 0-3 + 8-11 as u8 (slots 0..7); heads 4-7 as bf16
            cu8 = comb[:, [0, 1, 2, 3, 8, 9, 10, 11], :]
            att_c_in = np.ascontiguousarray(
                np.minimum(np.round(cu8), 255).astype(np.uint8)).reshape(128, 8192)
            att_b_in = np.ascontiguousarray(
                comb[:, 4:8, :].astype(bf)).reshape(128, 4096)
            # seq rows for this core's mention column half
            sq = seq_aug[g * 512:(g + 1) * 512]                   # [512, 770]
            sq = sq.reshape(4, 128, HA).transpose(1, 0, 2)        # [p, rq, 770]
            seq_in = np.ascontiguousarray(sq.astype(bf)).reshape(128, 4 * HA)
            # seqg rows at this core's span block positions
            sg = seq_aug[rowarr]                                  # [256, 770]
            sg = sg.reshape(2, 128, HA).transpose(1, 0, 2)        # [p, cc, 770]
            seqg_in = np.ascontiguousarray(sg.astype(bf)).reshape(128, 2 * HA)
            # masks [p, rq/cc, 8]
            mq = msb_g.reshape(2, 128, KH).transpose(1, 0, 2)
            msb_in = np.ascontiguousarray(mq.astype(bf)).reshape(128, 2 * KH)
            mcc_in = msb_in.copy()
            in_maps.append({"att_c": att_c_in, "att_b": att_b_in,
                            "seq": seq_in, "seqg": seqg_in,
                            "msb": msb_in, "mcc": mcc_in})
        metas.append((pos, (e - s).astype(np.float32)))
    return in_maps, metas


def _combine(outs, metas, sequence_output, type_table):
    seq = np.asarray(sequence_output, dtype=np.float32)
    ttab = np.asarray(type_table, dtype=np.float32)
    type_ids = np.concatenate(
        [np.zeros(E, np.int64), np.ones(EM, np.int64), np.full(K, 2, np.int64)])
    nodes_type = ttab[type_ids]                                   # [176, 20]

    out = np.zeros((B, E + EM + K + E + EM, H + TYPE_DIM), np.float32)
    for b in range(B):
        pos, length = metas[b]
        o0, o1 = outs[2 * b], outs[2 * b + 1]
        mnum = o0["out_m"].astype(np.float32) + o1["out_m"].astype(np.float32)
        v = np.concatenate([o0["out_v"], o1["out_v"]],
                           axis=0).astype(np.float32)             # [16, 770], x255

        m_ctx = mnum[:, :H] / (mnum[:, H:H + 1] + QS * NH * 1e-5)
        enum = mnum.reshape(E, MPE, HA).sum(axis=1)
        e_ctx = enum[:, :H] / (enum[:, H:H + 1] + QS * NH * MPE * 1e-5)
        link_rep = v[:, :H] / (QS * NH * length[:, None])

        memb = seq[b][pos]                                        # [128, 768] exact
        mg = memb.reshape(E, MPE, H)
        mmax = mg.max(axis=1)
        eemb = np.log(np.exp(mg - mmax[:, None, :]).sum(axis=1)) + mmax

        nodes_raw = np.concatenate([eemb, memb, link_rep], axis=0)    # [176, H]
        nodes = np.concatenate([nodes_raw, nodes_type], axis=1)       # [176, H+20]
        ctx = np.concatenate([e_ctx, m_ctx], axis=0)                  # [160, H]
        ctx = np.concatenate([ctx, np.zeros((E + EM, TYPE_DIM), np.float32)], axis=1)
        out[b] = np.concatenate([nodes, ctx], axis=0)
    return out


def kernel(**inputs):
    from concourse.bass_utils import run_bass_kernel_spmd

    in_maps, metas = _per_core_inputs(
        inputs["sequence_output"], inputs["attention"],
        inputs["mention_pos"], inputs["link_start"], inputs["link_len"])
    nc = _get_nc()
    res = run_bass_kernel_spmd(nc, in_maps, core_ids=list(range(8)))
    return _combine(res.results, metas, inputs["sequence_output"], inputs["type_table"])
